# revision 44
# baseline (speedup 1.0000x reference)
"""DGMC (deep graph matching consensus) Trainium2 kernel.

Data-parallel over the B=8 graph-pair batch: one graph pair per NeuronCore.
Per core:
  - Dense weighted adjacency A^T[s,d] (sum of edge_attr over edges s->d) is a
    host-side format conversion of (edge_index, edge_attr); all feature
    segment-sums run on device as A @ y tensor-engine matmuls streamed fp32r.
  - psi_1 GNN in feature-major layout -> h^T [64, 512]; S_hat0 = h_s^T h_t.
  - Per consensus step: softmax pieces (row-max / exp+row-sum / reciprocal),
    r_t = E^T (r_s/Z), o_t via A_t @ (r_t W2n), and mlp(D) via the packed-relu
    trick out[s,t] = sum_r w2[r] relu(a[s,r] - c[t,r]) computed as 128 fused
    DVE/ACT ops [128,512] (4 s-rows per op, bf16) + 128 PE matmuls with
    block-diagonal Wm2 weight variants accumulating into 32-row PSUM regions.
  - bm2 is dropped (softmax is shift-invariant).
"""
from contextlib import ExitStack

import ml_dtypes  # noqa: F401
import numpy as np

# ---------------------------------------------------------------------------
# Workaround: this walrus build only accepts one sync-wait per TPB_CTRL
# instruction; split the TileContext exit-drain waits across SP nops.
import concourse.tile as tile
from concourse import mybir
from concourse.vector_clock import ScopedClock


def _patched_drain_and_barrier(self, tick_clock, wait_clock):
    nop0 = self.nc.sync.nop(nofuse=True)
    wait_clock.add_sem_waits(nop0.ins, ScopedClock({None: tick_clock.global_clock}))
    si = nop0.ins.sync_info
    if si is not None and len(si.on_wait) > 1:
        waits = list(si.on_wait)
        nop0.ins.sync_info = mybir.SyncInfo(on_wait=waits[:1], on_update=list(si.on_update))
        for i in range(1, len(waits)):
            nop = self.nc.sync.nop(nofuse=True)
            nop.ins.sync_info = mybir.SyncInfo(on_wait=waits[i:i + 1], on_update=[])
    self.nc.sync.drain()
    self.nc.all_engine_barrier()
    assert self.sems is not None
    popped = self.nc._tile_sem_poison_stack.pop()
    assert popped is self._sem_poison
    self.nc.clear_and_free_semaphores(list(self.sems.allocated().values()))
    self.nc.all_engine_barrier()


tile.TileContext._drain_and_barrier = _patched_drain_and_barrier
# ---------------------------------------------------------------------------

import concourse.bacc as bacc

F32 = mybir.dt.float32
BF16 = mybir.dt.bfloat16
FP16 = mybir.dt.float16
F32R = mybir.dt.float32r

B = 8            # graph pairs (one per core)
N = 512          # nodes per graph
E = 8192         # edges per graph
DIN = 128
DH = 64
R = 32
NB = 4
STEPS = 2
N_CORES = 8


def build_kernel(repeats=1):
    nc = bacc.Bacc("TRN2", dynamic_dma_scratch_size=32768)

    xsT = nc.declare_dram_parameter("xsT", [DIN, N], F32, isOutput=False)
    xtT = nc.declare_dram_parameter("xtT", [DIN, N], F32, isOutput=False)
    rs_rows = nc.declare_dram_parameter("rs_rows", [128, STEPS, NB, R], F32, isOutput=False)
    rsT = nc.declare_dram_parameter("rsT", [STEPS, R, N], F32R, isOutput=False)
    A_in = nc.declare_dram_parameter("A_rows", [2, 128, NB, N], F32, isOutput=False)
    W1r = nc.declare_dram_parameter("W1r", [DIN, DH], F32, isOutput=False)
    W1n = nc.declare_dram_parameter("W1n", [DIN, DH], F32, isOutput=False)
    b1 = nc.declare_dram_parameter("b1", [DH, 1], F32, isOutput=False)
    W2r = nc.declare_dram_parameter("W2r", [R, R], F32R, isOutput=False)
    W2n = nc.declare_dram_parameter("W2n", [R, R], F32R, isOutput=False)
    b2 = nc.declare_dram_parameter("b2", [R, 1], F32, isOutput=False)
    Wm1 = nc.declare_dram_parameter("Wm1", [R, R], F32R, isOutput=False)
    bm1 = nc.declare_dram_parameter("bm1", [R, 1], F32, isOutput=False)
    W2p = nc.declare_dram_parameter("W2p", [128, 8, 128], FP16, isOutput=False)
    ident = nc.declare_dram_parameter("ident", [128, 128], F32R, isOutput=False)
    S0 = nc.declare_dram_parameter("S0", [N, N], F32, isOutput=True)
    SL = nc.declare_dram_parameter("SL", [N, N], F32, isOutput=True)

    with tile.TileContext(nc) as tc, ExitStack() as ctx:
        const = ctx.enter_context(tc.tile_pool(name="const", bufs=1))
        work = ctx.enter_context(tc.tile_pool(name="work", bufs=2))
        r4p = ctx.enter_context(tc.tile_pool(name="r4p", bufs=6))
        pbig = ctx.enter_context(tc.tile_pool(name="pbig", bufs=4, space="PSUM"))
        psmall = ctx.enter_context(tc.tile_pool(name="psmall", bufs=4, space="PSUM"))

        # load order: small compute-critical params first, big/late tensors last
        w1n = const.tile([DIN, DH], F32); nc.sync.dma_start(w1n[:], W1n[:])
        w1r = const.tile([DIN, DH], F32); nc.sync.dma_start(w1r[:], W1r[:])
        b1c = const.tile([DH, 1], F32); nc.sync.dma_start(b1c[:], b1[:])
        xs = const.tile([DIN, N], F32); nc.sync.dma_start(xs[:], xsT[:])
        xt = const.tile([DIN, N], F32); nc.sync.dma_start(xt[:], xtT[:])
        # warm the ACT function table immediately (costs ~2.7us once)
        actwarm = const.tile([DH, 1], F32)
        nc.scalar.activation(actwarm[:], b1c[:], mybir.ActivationFunctionType.Relu)
        w2r = const.tile([R, R], F32R); nc.sync.dma_start(w2r[:], W2r[:])
        w2n = const.tile([R, R], F32R); nc.sync.dma_start(w2n[:], W2n[:])
        b2c = const.tile([R, 1], F32); nc.sync.dma_start(b2c[:], b2[:])
        wm1 = const.tile([R, R], F32R); nc.sync.dma_start(wm1[:], Wm1[:])
        bm1c = const.tile([R, 1], F32); nc.sync.dma_start(bm1c[:], bm1[:])
        rst0 = const.tile([R, N], F32R); nc.sync.dma_start(rst0[:], rsT[0])
        rst1 = const.tile([R, N], F32R); nc.sync.dma_start(rst1[:], rsT[1])
        rsr = const.tile([128, STEPS, NB, R], F32)
        nc.sync.dma_start(rsr[:], rs_rows[:])
        rst_k = [rst0, rst1]
        rsr_k = [rsr[:, 0], rsr[:, 1]]

        A_sb, A_sbr = [], []
        for d in range(2):
            asb = const.tile([128, NB, N], F32, name=f"A_sb{d}")
            for sc_ in range(NB):
                nc.sync.dma_start(asb[:, sc_, :], A_in[d, :, sc_, :])
            A_sb.append(asb)
            asbr = const.tile([128, NB, N], F32R, name=f"A_sbr{d}")
            nc.scalar.copy(asbr[:], asb[:])
            A_sbr.append(asbr)
        A_s, A_t = A_sb
        A_sr, A_tr = A_sbr
        idn = const.tile([128, 128], F32R); nc.sync.dma_start(idn[:], ident[:])
        w2p = const.tile([128, 8, 128], FP16)
        nc.sync.dma_start(w2p[:], W2p[:])

        def add_aggT(ps, A, y_rows, cols, stop=True):
            """ps[f, d] += sum_s y[s, f] A^T[s, d] (agg arrives transposed)."""
            for sc in range(NB):
                nc.tensor.matmul(
                    ps[:], lhsT=y_rows[:, sc, cols], rhs=A[:, sc, :],
                    start=False, stop=(stop and sc == NB - 1),
                    skip_group_check=True)
            return ps

        def rows_from_psums(psums, Fdim, name, dt=F32R):
            t = work.tile([128, NB, Fdim], dt, tag=name, name=name)
            for db in range(NB):
                nc.vector.tensor_copy(t[:, db, :], psums[db][:])
            return t

        def body():
            # -------- psi_1 --------
            def psi1(xT, A, name):
                yps = []
                for nb_ in range(NB):
                    ps = psmall.tile([128, DH], F32, tag="small", name=f"y{name}{nb_}")
                    nc.tensor.matmul(ps[:], lhsT=xT[:, nb_ * 128:(nb_ + 1) * 128],
                                     rhs=w1n[:], start=True, stop=True)
                    yps.append(ps)
                y_rows = rows_from_psums(yps, DH, f"y{name}_rows", dt=F32)
                hps = psmall.tile([DH, N], F32, tag="small", name=f"h{name}ps")
                nc.tensor.matmul(hps[:], lhsT=w1r[:], rhs=xT[:], start=True, stop=False,
                                 skip_group_check=True)
                add_aggT(hps, A, y_rows, slice(0, DH))
                h = work.tile([DH, N], F32, tag=f"h{name}", name=f"h{name}")
                nc.scalar.activation(h[:], hps[:], mybir.ActivationFunctionType.Relu,
                                     bias=b1c[:, :1])
                return h

            h_s = psi1(xs, A_s, "s")
            h_t = psi1(xt, A_t, "t")

            # -------- S_hat0 --------
            S_hat = const.tile([128, NB, N], F32, name="S_hat")
            for sb in range(NB):
                ps = pbig.tile([128, N], F32, tag="big", name=f"sh0{sb}")
                nc.tensor.matmul(ps[:], lhsT=h_s[:, sb * 128:(sb + 1) * 128],
                                 rhs=h_t[:], start=True, stop=True)
                nc.vector.tensor_copy(S_hat[:, sb, :], ps[:])

            # -------- psi_2 graph-s (both steps, upfront) --------
            y0ps = []
            for k in range(STEPS):
                for nb_ in range(NB):
                    ps = psmall.tile([128, R], F32, tag="small", name=f"y0_{k}{nb_}")
                    nc.tensor.matmul(ps[:], lhsT=rst_k[k][:, nb_ * 128:(nb_ + 1) * 128],
                                     rhs=w2n[:], start=True, stop=True)
                    y0ps.append(ps)
            y0packed = work.tile([128, NB, 2 * R], F32R, tag="y0packed", name="y0packed")
            for k in range(STEPS):
                for nb_ in range(NB):
                    nc.vector.tensor_copy(y0packed[:, nb_, k * R:(k + 1) * R],
                                          y0ps[k * NB + nb_][:])
            A_packed = []
            for k in range(STEPS):
                osps = psmall.tile([R, N], F32, tag="small", name=f"osps{k}")
                nc.tensor.matmul(osps[:], lhsT=w2r[:], rhs=rst_k[k][:],
                                 start=True, stop=False, skip_group_check=True)
                add_aggT(osps, A_sr, y0packed, slice(k * R, (k + 1) * R))
                o_sT = work.tile([R, N], F32R, tag="o_sT", name=f"o_sT{k}")
                nc.scalar.activation(o_sT[:], osps[:],
                                     mybir.ActivationFunctionType.Relu, bias=b2c[:, :1])
                aps = psmall.tile([R, N], F32, tag="small", name=f"aT{k}ps")
                nc.tensor.matmul(aps[:], lhsT=wm1[:], rhs=o_sT[:],
                                 start=True, stop=True)
                aT = work.tile([R, N], F32, tag=f"aT{k}", name=f"aT{k}")
                nc.scalar.activation(aT[:], aps[:], mybir.ActivationFunctionType.Identity,
                                     bias=bm1c[:, :1])
                ap_t = const.tile([128, 128], F32, name=f"A_packed{k}")
                for j in range(4):
                    nc.vector.tensor_copy(
                        ap_t[R * j:R * (j + 1), :],
                        aT[:].rearrange("r (m j) -> r j m", j=4)[:, j, :])
                A_packed.append(ap_t)

            # -------- consensus steps --------
            mx = work.tile([128, NB], F32, tag="mx", name="mx")
            Z = work.tile([128, NB], F32, tag="Z", name="Z")
            rz = work.tile([128, NB], F32, tag="rz", name="rz")
            for k in range(STEPS):
                S_exp = work.tile([128, NB, N], F32, tag="S_exp", name=f"S_exp{k}")
                for sb in range(NB):
                    nc.vector.tensor_reduce(mx[:, sb:sb + 1], S_hat[:, sb, :],
                                            axis=mybir.AxisListType.X,
                                            op=mybir.AluOpType.max, negate=True)
                    nc.scalar.activation(S_exp[:, sb, :], S_hat[:, sb, :],
                                         mybir.ActivationFunctionType.Exp,
                                         bias=mx[:, sb:sb + 1],
                                         accum_out=Z[:, sb:sb + 1])
                nc.vector.reciprocal(rz[:], Z[:])
                if k == 0:
                    S0_sb = work.tile([128, NB, N], F32, tag="S0_sb", name="S0_sb")
                    for sb in range(NB):
                        nc.vector.tensor_scalar(
                            out=S0_sb[:, sb, :], in0=S_exp[:, sb, :],
                            scalar1=rz[:, sb:sb + 1], scalar2=None,
                            op0=mybir.AluOpType.mult)
                    nc.sync.dma_start(S0.rearrange("(a b) t -> b a t", b=128), S0_sb[:])
                rsp = work.tile([128, NB, R], F32, tag="rsp", name=f"rsp{k}")
                for sb in range(NB):
                    nc.vector.tensor_scalar(
                        out=rsp[:, sb, :], in0=rsr_k[k][:, sb, :],
                        scalar1=rz[:, sb:sb + 1], scalar2=None,
                        op0=mybir.AluOpType.mult)
                rtps = []
                for tb in range(NB):
                    ps = psmall.tile([128, R], F32, tag="small", name=f"rt{k}{tb}")
                    for sb in range(NB):
                        nc.tensor.matmul(ps[:], lhsT=S_exp[:, sb, tb * 128:(tb + 1) * 128],
                                         rhs=rsp[:, sb, :], start=(sb == 0),
                                         stop=(sb == NB - 1))
                    rtps.append(ps)
                rt_rows = rows_from_psums(rtps, R, "rt_rows")
                rtT_ps = psmall.tile([R, N], F32R, tag="small", name=f"rtTps{k}")
                for db in range(NB):
                    nc.tensor.matmul(
                        rtT_ps[:, db * 128:(db + 1) * 128], lhsT=rt_rows[:, db, :R],
                        rhs=idn[:], is_transpose=True, start=True, stop=True,
                        skip_group_check=True)
                rtT = work.tile([R, N], F32R, tag="rtT", name=f"rtT{k}")
                nc.scalar.copy(rtT[:], rtT_ps[:])
                y1ps = []
                for nb_ in range(NB):
                    ps = psmall.tile([128, R], F32, tag="small", name=f"y1_{k}{nb_}")
                    nc.tensor.matmul(ps[:], lhsT=rtT[:, nb_ * 128:(nb_ + 1) * 128],
                                     rhs=w2n[:], start=True, stop=True)
                    y1ps.append(ps)
                y1_rows = rows_from_psums(y1ps, R, "y1_rows")
                otps = psmall.tile([R, N], F32, tag="small", name=f"otT{k}")
                nc.tensor.matmul(otps[:], lhsT=w2r[:], rhs=rtT[:], start=True, stop=False,
                                 skip_group_check=True)
                add_aggT(otps, A_tr, y1_rows, slice(0, R))
                o_tT = work.tile([R, N], F32R, tag="o_tT", name=f"o_tT{k}")
                nc.scalar.activation(o_tT[:], otps[:], mybir.ActivationFunctionType.Relu,
                                     bias=b2c[:, :1])
                cps = psmall.tile([R, N], F32, tag="small", name=f"cT{k}")
                nc.tensor.matmul(cps[:], lhsT=wm1[:], rhs=o_tT[:], start=True, stop=True)
                cT4neg = work.tile([128, N], FP16, tag="cT4neg", name=f"cT4neg{k}")
                for j in range(4):
                    nc.scalar.mul(cT4neg[R * j:R * (j + 1), :], cps[:], -1.0)
                mlp_ps = [pbig.tile([128, N], F32, tag="big", name=f"mlp{k}{b_}")
                          for b_ in range(NB)]
                for m in range(128):
                    r4 = r4p.tile([128, N], FP16, tag="r4")
                    if m % 4 != 3:
                        nc.vector.tensor_scalar(
                            out=r4[:], in0=cT4neg[:],
                            scalar1=A_packed[k][:, m:m + 1], scalar2=0.0,
                            op0=mybir.AluOpType.add, op1=mybir.AluOpType.max)
                    else:
                        nc.scalar.activation(r4[:], cT4neg[:],
                                             mybir.ActivationFunctionType.Relu,
                                             bias=A_packed[k][:, m:m + 1])
                    blk, grp, v = m // 32, (m // 8) % 4, m % 8
                    if grp < 2:
                        nc.tensor.matmul(mlp_ps[blk][32 * grp:32 * (grp + 1), :],
                                         lhsT=w2p[:, v, 0:32], rhs=r4[:],
                                         start=(v == 0), stop=(v == 7))
                    elif grp == 2:
                        nc.tensor.matmul(mlp_ps[blk][64:128, :],
                                         lhsT=w2p[:, v, 0:64], rhs=r4[:],
                                         start=(v == 0), stop=False)
                    else:
                        nc.tensor.matmul(mlp_ps[blk][64:128, :],
                                         lhsT=w2p[:, v, 64:128], rhs=r4[:],
                                         start=False, stop=(v == 7))
                for sb in range(NB):
                    nc.vector.tensor_tensor(out=S_hat[:, sb, :], in0=S_hat[:, sb, :],
                                            in1=mlp_ps[sb][:], op=mybir.AluOpType.add)

            # -------- final softmax -> SL --------
            SL_sb = work.tile([128, NB, N], F32, tag="S0_sb", name="SL_sb")
            for sb in range(NB):
                nc.vector.tensor_reduce(mx[:, sb:sb + 1], S_hat[:, sb, :],
                                        axis=mybir.AxisListType.X,
                                        op=mybir.AluOpType.max, negate=True)
                nc.scalar.activation(SL_sb[:, sb, :], S_hat[:, sb, :],
                                     mybir.ActivationFunctionType.Exp,
                                     bias=mx[:, sb:sb + 1], accum_out=Z[:, sb:sb + 1])
            nc.vector.reciprocal(rz[:], Z[:])
            SLv = SL.rearrange("(a b) t -> a b t", b=128)
            for sb in range(NB):
                nc.vector.tensor_scalar(out=SL_sb[:, sb, :], in0=SL_sb[:, sb, :],
                                        scalar1=rz[:, sb:sb + 1], scalar2=None,
                                        op0=mybir.AluOpType.mult)
                nc.sync.dma_start(SLv[sb], SL_sb[:, sb, :])

        if repeats == 1:
            body()
        else:
            with tc.For_i(0, repeats, 1):
                body()

    nc.compile()
    return nc


def prep_core_inputs(g, inp):
    n0 = g * N
    xs = np.asarray(inp["x_s"][n0:n0 + N], np.float32)
    xt = np.asarray(inp["x_t"][n0:n0 + N], np.float32)
    es = np.asarray(inp["edge_index_s"][:, g * E:(g + 1) * E]).astype(np.int64) - n0
    et = np.asarray(inp["edge_index_t"][:, g * E:(g + 1) * E]).astype(np.int64) - n0
    eas = np.asarray(inp["edge_attr_s"][g * E:(g + 1) * E, 0], np.float64)
    eat = np.asarray(inp["edge_attr_t"][g * E:(g + 1) * E, 0], np.float64)
    rs = np.asarray(inp["r_s_all"][:, g], np.float32)

    def a_build(edges, ea):
        at = np.bincount(edges[0] * N + edges[1], weights=ea, minlength=N * N)
        at = at.astype(np.float32).reshape(NB, 128, N).transpose(1, 0, 2)
        return np.ascontiguousarray(at)

    w2 = np.asarray(inp["Wm2"], np.float32)[:, 0]
    w2p = np.zeros((128, 8, 128), np.float32)
    for v in range(8):
        for j in range(4):
            w2p[R * j:R * (j + 1), v, 4 * v + j] = w2
            w2p[R * j:R * (j + 1), v, 96 + 4 * v + j] = w2
    return {
        "xsT": np.ascontiguousarray(xs.T),
        "xtT": np.ascontiguousarray(xt.T),
        "rs_rows": np.ascontiguousarray(rs.reshape(STEPS, NB, 128, R).transpose(2, 0, 1, 3)),
        "rsT": np.ascontiguousarray(rs.transpose(0, 2, 1)),
        "A_rows": np.stack([a_build(es, eas), a_build(et, eat)]),
        "W1r": np.asarray(inp["W1_root"], np.float32),
        "W1n": np.asarray(inp["W1_nbr"], np.float32),
        "b1": np.asarray(inp["b1"], np.float32).reshape(DH, 1),
        "W2r": np.asarray(inp["W2_root"], np.float32),
        "W2n": np.asarray(inp["W2_nbr"], np.float32),
        "b2": np.asarray(inp["b2"], np.float32).reshape(R, 1),
        "Wm1": np.asarray(inp["Wm1"], np.float32),
        "bm1": np.asarray(inp["bm1"], np.float32).reshape(R, 1),
        "W2p": w2p.astype(np.float16),
        "ident": np.eye(128, dtype=np.float32),
    }


_NC_CACHE = {}


def _get_nc(repeats=1):
    if repeats not in _NC_CACHE:
        _NC_CACHE[repeats] = build_kernel(repeats)
    return _NC_CACHE[repeats]


def kernel(**inputs):
    from concourse.bass_utils import run_bass_kernel_spmd
    nc = _get_nc(1)
    in_maps = [prep_core_inputs(g, inputs) for g in range(B)]
    res = run_bass_kernel_spmd(nc, in_maps, core_ids=list(range(N_CORES)))
    S0 = np.stack([res.results[g]["S0"] for g in range(B)])
    SL = np.stack([res.results[g]["SL"] for g in range(B)])
    return S0, SL



# revision 48
# speedup vs baseline: 1.2997x; 1.2997x over previous
"""DGMC (deep graph matching consensus) Trainium2 kernel, v2.

Data-parallel over the B=8 graph-pair batch: one graph pair per NeuronCore.
Per core:
  - Dense weighted adjacency A^T[s,d] is a host-side format conversion of
    (edge_index, edge_attr); feature segment-sums run as A @ y matmuls (f32r).
  - psi_1 in feature-major layout -> h^T [64, 512] f32r; S_hat0 = h_s^T h_t.
  - r_t^T computed directly as sum_sb (rsp_sb)^T @ S_exp_sb (no transposes).
  - -c^T replicated 4x via a single matmul with a stacked negated Wm1.
  - mlp(D) via the packed-relu trick: out[s,t] = sum_r w2[r] relu(a[s,r]-c[t,r])
    computed as 128 [128,512] fp16 relu ops split DVE/ACT/Pool + 128 PE matmuls
    with block-diagonal Wm2 variants accumulating into 32-row PSUM regions.
  - Softmax/consensus chains for step k+1 are interleaved per-block into step
    k's mlp loop; exp shifts reuse prev max + max(mlp block) (softmax is
    shift-invariant, so any safe shift is exact).
  - bm2 dropped (softmax shift-invariant).
"""
from contextlib import ExitStack

import ml_dtypes  # noqa: F401
import numpy as np

# ---------------------------------------------------------------------------
# Workaround: this walrus build only accepts one sync-wait per TPB_CTRL
# instruction; split the TileContext exit-drain waits across SP nops.
import concourse.tile as tile
from concourse import mybir
from concourse.vector_clock import ScopedClock


def _patched_drain_and_barrier(self, tick_clock, wait_clock):
    nop0 = self.nc.sync.nop(nofuse=True)
    wait_clock.add_sem_waits(nop0.ins, ScopedClock({None: tick_clock.global_clock}))
    si = nop0.ins.sync_info
    if si is not None and len(si.on_wait) > 1:
        waits = list(si.on_wait)
        nop0.ins.sync_info = mybir.SyncInfo(on_wait=waits[:1], on_update=list(si.on_update))
        for i in range(1, len(waits)):
            nop = self.nc.sync.nop(nofuse=True)
            nop.ins.sync_info = mybir.SyncInfo(on_wait=waits[i:i + 1], on_update=[])
    self.nc.sync.drain()
    self.nc.all_engine_barrier()
    assert self.sems is not None
    popped = self.nc._tile_sem_poison_stack.pop()
    assert popped is self._sem_poison
    self.nc.clear_and_free_semaphores(list(self.sems.allocated().values()))
    self.nc.all_engine_barrier()


tile.TileContext._drain_and_barrier = _patched_drain_and_barrier
# ---------------------------------------------------------------------------

import concourse.bacc as bacc

F32 = mybir.dt.float32
FP16 = mybir.dt.float16
F32R = mybir.dt.float32r

B = 8            # graph pairs (one per core)
N = 512          # nodes per graph
E = 8192         # edges per graph
DIN = 128
DH = 64
R = 32
NB = 4
STEPS = 2
N_CORES = 8

# pblob32 column layout
PB_W2R = 0
PB_W2N = 32
PB_B2 = 64
PB_WM1 = 65
PB_WM1N4 = 97
PB_BM1 = 225
PB_RST0 = 226
PB_RST1 = 738
PB32_COLS = 1250

# pblob128 column layout
PB_W1N = 0
PB_W1R = 64
PB_B1 = 128
PB_RSR = 129
PB128_COLS = 129 + STEPS * NB * R

AX = mybir.AxisListType.X
MAX = mybir.AluOpType.max
ADD = mybir.AluOpType.add
MULT = mybir.AluOpType.mult
RELU = mybir.ActivationFunctionType.Relu
EXP = mybir.ActivationFunctionType.Exp
IDENT = mybir.ActivationFunctionType.Identity
COPY = mybir.ActivationFunctionType.Copy


def build_kernel(repeats=1):
    nc = bacc.Bacc("TRN2", dynamic_dma_scratch_size=32768)

    xsT = nc.declare_dram_parameter("xsT", [DIN, N], F32, isOutput=False)
    xtT = nc.declare_dram_parameter("xtT", [DIN, N], F32, isOutput=False)
    A_in = nc.declare_dram_parameter("A_rows", [2, 128, NB * N], F32, isOutput=False)
    PB128 = nc.declare_dram_parameter("pblob128", [128, PB128_COLS], F32, isOutput=False)
    PB32 = nc.declare_dram_parameter("pblob32", [R, PB32_COLS], F32R, isOutput=False)
    W2p = nc.declare_dram_parameter("W2p", [128, 8, 128], FP16, isOutput=False)
    S0 = nc.declare_dram_parameter("S0", [N, N], F32, isOutput=True)
    SL = nc.declare_dram_parameter("SL", [N, N], F32, isOutput=True)

    with tile.TileContext(nc) as tc, ExitStack() as ctx:
        const = ctx.enter_context(tc.tile_pool(name="const", bufs=1))
        work = ctx.enter_context(tc.tile_pool(name="work", bufs=2))
        souts = ctx.enter_context(tc.tile_pool(name="souts", bufs=4))
        r4p = ctx.enter_context(tc.tile_pool(name="r4p", bufs=14))
        pbig = ctx.enter_context(tc.tile_pool(name="pbig", bufs=4, space="PSUM"))
        psmall = ctx.enter_context(tc.tile_pool(name="psmall", bufs=2, space="PSUM"))
        prtT = ctx.enter_context(tc.tile_pool(name="prtT", bufs=2, space="PSUM"))

        # ---- input DMAs: A_s on the ACT queue, the rest on SP in arrival-
        # priority order (A_t after xt, param blobs around them) ----
        A_s = const.tile([128, NB, N], F32, name="A_s")
        nc.scalar.dma_start(A_s[:].rearrange("p b n -> p (b n)"), A_in[0])
        pb128 = const.tile([128, PB128_COLS], F32)
        nc.sync.dma_start(pb128[:], PB128[:])
        xs = const.tile([DIN, N], F32); nc.sync.dma_start(xs[:], xsT[:])
        xt = const.tile([DIN, N], F32); nc.sync.dma_start(xt[:], xtT[:])
        A_t = const.tile([128, NB, N], F32, name="A_t")
        nc.sync.dma_start(A_t[:].rearrange("p b n -> p (b n)"), A_in[1])
        pb32 = const.tile([R, PB32_COLS], F32R)
        nc.sync.dma_start(pb32[:], PB32[:])
        w2p = const.tile([128, 8, 128], FP16)
        nc.sync.dma_start(w2p[:], W2p[:])
        # f32r-rounded copies of A for the precision-tolerant psi_2 aggs
        # (one-time, outside the repeat body; walrus requires a rounding op)
        A_sr_t = const.tile([128, NB, N], F32R, name="A_sr")
        nc.scalar.copy(A_sr_t[:], A_s[:])
        A_tr_t = const.tile([128, NB, N], F32R, name="A_tr")
        nc.vector.tensor_copy(A_tr_t[:], A_t[:])
        A_sr = A_sr_t[:]
        A_tr = A_tr_t[:]

        # named parameter views
        w1n = pb128[:, PB_W1N:PB_W1N + DH]
        w1r = pb128[:, PB_W1R:PB_W1R + DH]
        b1c = pb128[0:DH, PB_B1:PB_B1 + 1]
        rsr = pb128[:, PB_RSR:PB_RSR + STEPS * NB * R].rearrange(
            "p (k b r) -> p k b r", k=STEPS, b=NB)
        w2r = pb32[:, PB_W2R:PB_W2R + R]
        w2n = pb32[:, PB_W2N:PB_W2N + R]
        b2c = pb32[:, PB_B2:PB_B2 + 1]
        wm1 = pb32[:, PB_WM1:PB_WM1 + R]
        wm1n4 = pb32[:, PB_WM1N4:PB_WM1N4 + 128]
        bm1c = pb32[:, PB_BM1:PB_BM1 + 1]
        rst = [pb32[:, PB_RST0:PB_RST0 + N], pb32[:, PB_RST1:PB_RST1 + N]]

        # warm the ACT function table early (costs ~1.3us once)
        actwarm = const.tile([DH, 1], F32)
        nc.scalar.activation(actwarm[:], b1c, RELU)

        # persistent state tiles
        S_hat = const.tile([128, NB, N], F32, name="S_hat")
        mx0 = const.tile([128, NB], F32, name="mx0")     # -(max of S_hat0)
        sh1 = const.tile([128, NB], F32, name="sh1")     # -(shift for step-1 softmax)
        shF = const.tile([128, NB], F32, name="shF")     # -(shift for final softmax)
        Z0 = const.tile([128, NB], F32, name="Z0")
        Z1 = const.tile([128, NB], F32, name="Z1")
        ZF = const.tile([128, NB], F32, name="ZF")
        rz0 = const.tile([128, NB], F32, name="rz0")
        rz1 = const.tile([128, NB], F32, name="rz1")
        rzF = const.tile([128, NB], F32, name="rzF")
        rsp0 = const.tile([128, NB, R], F32R, name="rsp0")
        rsp1 = const.tile([128, NB, R], F32R, name="rsp1")
        A_packed = [const.tile([128, 128], F32, name=f"A_packed{k}") for k in range(STEPS)]
        cT4neg = [const.tile([128, N], FP16, name=f"cT4neg{k}") for k in range(STEPS)]

        S0v = S0.rearrange("(a b) t -> a b t", b=128)
        SLv = SL.rearrange("(a b) t -> a b t", b=128)

        def add_aggT(ps, A, y_rows, cols, stop=True):
            """ps[f, d] += sum_s y[s, f] A^T[s, d] (agg arrives transposed)."""
            for sc in range(NB):
                nc.tensor.matmul(
                    ps[:], lhsT=y_rows[:, sc, cols], rhs=A[:, sc, :],
                    start=False, stop=(stop and sc == NB - 1),
                    skip_group_check=True)
            return ps

        def psi2s_stages(k):
            """o_s -> a -> A_packed for step k (the s-graph side of psi_2).

            Returns a list of stage closures so the caller can spread the
            issue points (per-engine streams dispatch in order; a stage
            issued too early head-of-line-blocks that engine)."""
            box = {}

            def s1():  # PE: y0 = rs^T W2n
                box["ypk"] = psmall.tile([128, NB, R], F32, tag="small",
                                         name=f"y0pk{k}")
                for nb_ in range(NB):
                    nc.tensor.matmul(box["ypk"][:, nb_, :],
                                     lhsT=rst[k][:, nb_ * 128:(nb_ + 1) * 128],
                                     rhs=w2n, start=True, stop=True,
                                     skip_group_check=True)

            def s2():  # DVE: psum -> sbuf
                box["y0rows"] = work.tile([128, NB, R], F32R, tag="y0rows",
                                          name=f"y0rows{k}")
                nc.vector.tensor_copy(box["y0rows"][:], box["ypk"][:])

            def s3():  # PE: o_s psum = W2r^T rs + A_s-agg
                box["osps"] = psmall.tile([R, N], F32, tag="small", name=f"osps{k}")
                nc.tensor.matmul(box["osps"][:], lhsT=w2r, rhs=rst[k], start=True,
                                 stop=False, skip_group_check=True)
                add_aggT(box["osps"], A_sr, box["y0rows"], slice(0, R))

            def s4():  # ACT: relu -> o_sT
                box["o_sT"] = work.tile([R, N], F32R, tag="o_sT", name=f"o_sT{k}")
                nc.scalar.activation(box["o_sT"][:], box["osps"][:], RELU, bias=b2c)

            def s5():  # PE: a psum = Wm1^T o_s
                box["aps"] = psmall.tile([R, N], F32, tag="small", name=f"aps{k}")
                nc.tensor.matmul(box["aps"][:], lhsT=wm1, rhs=box["o_sT"][:],
                                 start=True, stop=True, skip_group_check=True)

            def s6():  # ACT: + bm1 -> aT
                box["aT"] = work.tile([R, N], F32, tag="aT", name=f"aT{k}")
                nc.scalar.activation(box["aT"][:], box["aps"][:], IDENT, bias=bm1c)

            def s7():  # DVE: rearrange into A_packed
                av = box["aT"][:].rearrange("r (m j) -> r j m", j=4)
                for j in range(4):
                    nc.vector.tensor_copy(A_packed[k][R * j:R * (j + 1), :],
                                          av[:, j, :])

            return [s1, s2, s3, s4, s5, s6, s7]

        def psi2t_tail(k, rtT_ps):
            """rt -> o_t -> -c^T(4x) for step k, after rtT accumulation."""
            rtT = work.tile([R, N], F32R, tag="rtT", name=f"rtT{k}")
            nc.scalar.copy(rtT[:], rtT_ps[:])
            y1pk = psmall.tile([128, NB, R], F32, tag="small", name=f"y1pk{k}")
            for nb_ in range(NB):
                nc.tensor.matmul(y1pk[:, nb_, :], lhsT=rtT[:, nb_ * 128:(nb_ + 1) * 128],
                                 rhs=w2n, start=True, stop=True, skip_group_check=True)
            y1rows = work.tile([128, NB, R], F32R, tag="y0rows", name=f"y1rows{k}")
            nc.vector.tensor_copy(y1rows[:], y1pk[:])
            otps = prtT.tile([R, N], F32, tag="rtT", name=f"otps{k}")
            nc.tensor.matmul(otps[:], lhsT=w2r, rhs=rtT[:], start=True, stop=False,
                             skip_group_check=True)
            add_aggT(otps, A_tr, y1rows, slice(0, R))
            o_tT = work.tile([R, N], F32R, tag="o_sT", name=f"o_tT{k}")
            nc.scalar.activation(o_tT[:], otps[:], RELU, bias=b2c)
            cps4 = pbig.tile([128, N], F32, tag="big", name=f"cps4_{k}")
            nc.tensor.matmul(cps4[:], lhsT=wm1n4, rhs=o_tT[:], start=True, stop=True,
                             skip_group_check=True)
            nc.scalar.copy(cT4neg[k][:], cps4[:])

        def body():
            # -------- psi_1 --------
            def psi1(xT, A, name):
                ypk = psmall.tile([128, NB, DH], F32, tag="small", name=f"y{name}pk")
                for nb_ in range(NB):
                    nc.tensor.matmul(ypk[:, nb_, :], lhsT=xT[:, nb_ * 128:(nb_ + 1) * 128],
                                     rhs=w1n, start=True, stop=True, skip_group_check=True)
                y_rows = work.tile([128, NB, DH], F32, tag=f"y{name}_rows",
                                   name=f"y{name}_rows")
                nc.vector.tensor_copy(y_rows[:], ypk[:])
                hps = pbig.tile([DH, N], F32, tag="big", name=f"h{name}ps")
                nc.tensor.matmul(hps[:], lhsT=w1r, rhs=xT[:], start=True, stop=False,
                                 skip_group_check=True)
                add_aggT(hps, A, y_rows, slice(0, DH))
                # h kept fp32: the S_hat0 contraction amplifies h/matmul error
                # by ~|S_hat| (exp sensitivity), f32r there costs ~1.6e-2 rel
                h = work.tile([DH, N], F32, tag=f"h{name}", name=f"h{name}")
                nc.scalar.activation(h[:], hps[:], RELU, bias=b1c)
                return h

            h_s = psi1(xs, A_s, "s")
            h_t = psi1(xt, A_t, "t")

            # -------- S_hat0 + softmax0 + rt0, per block --------
            S_exp0 = work.tile([128, NB, N], F32R, tag="S_exp", name="S_exp0")
            rtT0_ps = prtT.tile([R, N], F32, tag="rtT", name="rtT0ps")
            for sb in range(NB):
                ps = pbig.tile([128, N], F32, tag="big", name=f"sh0{sb}")
                nc.tensor.matmul(ps[:], lhsT=h_s[:, sb * 128:(sb + 1) * 128],
                                 rhs=h_t[:], start=True, stop=True, skip_group_check=True)
                # copy S_hat to SBUF (alternate DVE/ACT), then reduce and
                # exp from SBUF (PSUM-sourced ACT/DVE ops are slow on HW)
                if sb % 2 == 0:
                    nc.scalar.copy(S_hat[:, sb, :], ps[:])
                else:
                    nc.vector.tensor_copy(S_hat[:, sb, :], ps[:])
                nc.vector.tensor_reduce(mx0[:, sb:sb + 1], S_hat[:, sb, :], axis=AX,
                                        op=MAX, negate=True)
                nc.scalar.activation(S_exp0[:, sb, :], S_hat[:, sb, :], EXP,
                                     bias=mx0[:, sb:sb + 1],
                                     accum_out=Z0[:, sb:sb + 1])
                nc.vector.reciprocal(rz0[:, sb:sb + 1], Z0[:, sb:sb + 1])
                nc.vector.tensor_scalar(
                    out=rsp0[:, sb, :], in0=rsr[:, 0, sb, :],
                    scalar1=rz0[:, sb:sb + 1], scalar2=None, op0=MULT)

            # psi_2 s-side for step 0 (overlaps softmax0 on PE)
            for stage in psi2s_stages(0):
                stage()

            for sb in range(NB):
                nc.tensor.matmul(rtT0_ps[:], lhsT=rsp0[:, sb, :], rhs=S_exp0[:, sb, :],
                                 start=(sb == 0), stop=(sb == NB - 1),
                                 skip_group_check=True)
                # S0 output: exp * rz (split DVE/ACT), then DMA out
                sb_t = souts.tile([128, N], F32, tag="Sout", name=f"S0b{sb}")
                if sb % 2 == 0:
                    nc.vector.tensor_scalar(out=sb_t[:], in0=S_exp0[:, sb, :],
                                            scalar1=rz0[:, sb:sb + 1], scalar2=None,
                                            op0=MULT)
                else:
                    nc.scalar.activation(sb_t[:], S_exp0[:, sb, :], COPY,
                                         scale=rz0[:, sb:sb + 1])
                nc.sync.dma_start(S0v[sb], sb_t[:])

            psi2t_tail(0, rtT0_ps)

            # -------- consensus steps: mlp loop with interleaved chains --------
            S_exp1 = work.tile([128, NB, N], F32R, tag="S_exp", name="S_exp1")
            acc1_box = []

            def chain_a(k, b):
                """S_hat[b] += mlp_ps[b]; -(true row max) from SBUF (DVE)."""
                sh = sh1 if k == 0 else shF
                nc.vector.tensor_tensor(out=S_hat[:, b, :], in0=S_hat[:, b, :],
                                        in1=mlp_ps[b][:], op=ADD)
                nc.vector.tensor_reduce(sh[:, b:b + 1], S_hat[:, b, :], axis=AX,
                                        op=MAX, negate=True)

            def chain_b(k, b):
                if k == 0:
                    nc.scalar.activation(S_exp1[:, b, :], S_hat[:, b, :], EXP,
                                         bias=sh1[:, b:b + 1],
                                         accum_out=Z1[:, b:b + 1])
                    nc.vector.reciprocal(rz1[:, b:b + 1], Z1[:, b:b + 1])
                    nc.vector.tensor_scalar(
                        out=rsp1[:, b, :], in0=rsr[:, 1, b, :],
                        scalar1=rz1[:, b:b + 1], scalar2=None, op0=MULT)
                else:
                    sle = work.tile([128, N], F32, tag="SLe", name=f"SLe{b}")
                    nc.scalar.activation(sle[:], S_hat[:, b, :], EXP,
                                         bias=shF[:, b:b + 1],
                                         accum_out=ZF[:, b:b + 1])
                    nc.vector.reciprocal(rzF[:, b:b + 1], ZF[:, b:b + 1])
                    slb = souts.tile([128, N], F32, tag="Sout", name=f"SLb{b}")
                    nc.scalar.activation(slb[:], sle[:], COPY,
                                         scale=rzF[:, b:b + 1])
                    nc.sync.dma_start(SLv[b], slb[:])

            def chain_c(k, b):
                if k == 0:
                    nc.tensor.matmul(acc1_box[0][:], lhsT=rsp1[:, b, :],
                                     rhs=S_exp1[:, b, :], start=(b == 0),
                                     stop=(b == NB - 1), skip_group_check=True)

            for k in range(STEPS):
                if k == 0:
                    acc1_box.append(prtT.tile([R, N], F32, tag="rtT",
                                              name="rtT1ps"))
                    # deferred psi_2 s-side for step 1, staged across mlp0
                    stages1 = psi2s_stages(1)
                    # (issue m, stage): PE stages early, consumers offset past
                    # the producing engine's ring lag (~12 m's)
                    stage_at = {16: stages1[0], 34: stages1[1], 36: stages1[2],
                                38: stages1[3], 41: stages1[4], 44: stages1[5],
                                72: stages1[6]}
                else:
                    stage_at = {}
                mlp_ps = [pbig.tile([128, N], F32, tag="big", name=f"mlp{k}{b_}")
                          for b_ in range(NB)]
                for m in range(128):
                    r4 = r4p.tile([128, N], FP16, tag="r4")
                    r8 = m % 8
                    if r8 in (3, 7):
                        nc.scalar.activation(r4[:], cT4neg[k][:], RELU,
                                             bias=A_packed[k][:, m:m + 1])
                    else:
                        nc.vector.tensor_scalar(
                            out=r4[:], in0=cT4neg[k][:],
                            scalar1=A_packed[k][:, m:m + 1], scalar2=0.0,
                            op0=ADD, op1=MAX)
                    blk, grp, v = m // 32, (m // 8) % 4, m % 8
                    if grp < 2:
                        nc.tensor.matmul(mlp_ps[blk][32 * grp:32 * (grp + 1), :],
                                         lhsT=w2p[:, v, 0:32], rhs=r4[:],
                                         start=(v == 0), stop=(v == 7))
                    elif grp == 2:
                        nc.tensor.matmul(mlp_ps[blk][64:128, :],
                                         lhsT=w2p[:, v, 0:64], rhs=r4[:],
                                         start=(v == 0), stop=False)
                    else:
                        nc.tensor.matmul(mlp_ps[blk][64:128, :],
                                         lhsT=w2p[:, v, 64:128], rhs=r4[:],
                                         start=False, stop=(v == 7))
                    if m in stage_at:
                        stage_at[m]()
                    # interleaved per-block chains
                    if m >= 32:
                        mb, off = (m // 32) - 1, m % 32
                        if off == 8:
                            chain_a(k, mb)
                        elif off == 14:
                            chain_b(k, mb)
                        elif off == 18:
                            chain_c(k, mb)
                # tail: last block's chain
                chain_a(k, NB - 1)
                chain_b(k, NB - 1)
                chain_c(k, NB - 1)
                if k == 0:
                    psi2t_tail(1, acc1_box[0])

        if repeats == 1:
            body()
        else:
            with tc.For_i(0, repeats, 1):
                body()

    nc.compile()
    return nc


def prep_core_inputs(g, inp):
    n0 = g * N
    xs = np.asarray(inp["x_s"][n0:n0 + N], np.float32)
    xt = np.asarray(inp["x_t"][n0:n0 + N], np.float32)
    es = np.asarray(inp["edge_index_s"][:, g * E:(g + 1) * E]).astype(np.int64) - n0
    et = np.asarray(inp["edge_index_t"][:, g * E:(g + 1) * E]).astype(np.int64) - n0
    eas = np.asarray(inp["edge_attr_s"][g * E:(g + 1) * E, 0], np.float64)
    eat = np.asarray(inp["edge_attr_t"][g * E:(g + 1) * E, 0], np.float64)
    rs = np.asarray(inp["r_s_all"][:, g], np.float32)

    def a_build(edges, ea):
        at = np.bincount(edges[0] * N + edges[1], weights=ea, minlength=N * N)
        at = at.astype(np.float32).reshape(NB, 128, N).transpose(1, 0, 2)
        return np.ascontiguousarray(at.reshape(128, NB * N))

    w2 = np.asarray(inp["Wm2"], np.float32)[:, 0]
    w2p = np.zeros((128, 8, 128), np.float32)
    for v in range(8):
        for j in range(4):
            w2p[R * j:R * (j + 1), v, 4 * v + j] = w2
            w2p[R * j:R * (j + 1), v, 96 + 4 * v + j] = w2

    pb128 = np.zeros((128, PB128_COLS), np.float32)
    pb128[:, PB_W1N:PB_W1N + DH] = np.asarray(inp["W1_nbr"], np.float32)
    pb128[:, PB_W1R:PB_W1R + DH] = np.asarray(inp["W1_root"], np.float32)
    pb128[0:DH, PB_B1] = np.asarray(inp["b1"], np.float32)
    rsr = rs.reshape(STEPS, NB, 128, R).transpose(2, 0, 1, 3)  # [128, k, b, R]
    pb128[:, PB_RSR:] = rsr.reshape(128, STEPS * NB * R)

    pb32 = np.zeros((R, PB32_COLS), np.float32)
    pb32[:, PB_W2R:PB_W2R + R] = np.asarray(inp["W2_root"], np.float32)
    pb32[:, PB_W2N:PB_W2N + R] = np.asarray(inp["W2_nbr"], np.float32)
    pb32[:, PB_B2] = np.asarray(inp["b2"], np.float32)
    wm1 = np.asarray(inp["Wm1"], np.float32)
    pb32[:, PB_WM1:PB_WM1 + R] = wm1
    pb32[:, PB_WM1N4:PB_WM1N4 + 128] = np.tile(-wm1, (1, 4))
    pb32[:, PB_BM1] = np.asarray(inp["bm1"], np.float32)
    rstv = rs.transpose(0, 2, 1)  # [STEPS, R, N]
    pb32[:, PB_RST0:PB_RST0 + N] = rstv[0]
    pb32[:, PB_RST1:PB_RST1 + N] = rstv[1]

    return {
        "xsT": np.ascontiguousarray(xs.T),
        "xtT": np.ascontiguousarray(xt.T),
        "A_rows": np.stack([a_build(es, eas), a_build(et, eat)]),
        "pblob128": pb128,
        "pblob32": pb32,
        "W2p": w2p.astype(np.float16),
    }


_NC_CACHE = {}


def _get_nc(repeats=1):
    if repeats not in _NC_CACHE:
        _NC_CACHE[repeats] = build_kernel(repeats)
    return _NC_CACHE[repeats]


def kernel(**inputs):
    from concourse.bass_utils import run_bass_kernel_spmd
    nc = _get_nc(1)
    in_maps = [prep_core_inputs(g, inputs) for g in range(B)]
    res = run_bass_kernel_spmd(nc, in_maps, core_ids=list(range(N_CORES)))
    S0 = np.stack([res.results[g]["S0"] for g in range(B)])
    SL = np.stack([res.results[g]["SL"] for g in range(B)])
    return S0, SL


# revision 50
# speedup vs baseline: 3.0733x; 2.3647x over previous
"""DGMC (deep graph matching consensus) Trainium2 kernel, v2.

Data-parallel over the B=8 graph-pair batch: one graph pair per NeuronCore.
Per core:
  - Dense weighted adjacency A^T[s,d] is a host-side format conversion of
    (edge_index, edge_attr); feature segment-sums run as A @ y matmuls (f32r).
  - psi_1 in feature-major layout -> h^T [64, 512] f32r; S_hat0 = h_s^T h_t.
  - r_t^T computed directly as sum_sb (rsp_sb)^T @ S_exp_sb (no transposes).
  - -c^T replicated 4x via a single matmul with a stacked negated Wm1.
  - mlp(D) via the packed-relu trick: out[s,t] = sum_r w2[r] relu(a[s,r]-c[t,r])
    computed as 128 [128,512] fp16 relu ops split DVE/ACT/Pool + 128 PE matmuls
    with block-diagonal Wm2 variants accumulating into 32-row PSUM regions.
  - Softmax/consensus chains for step k+1 are interleaved per-block into step
    k's mlp loop; exp shifts reuse prev max + max(mlp block) (softmax is
    shift-invariant, so any safe shift is exact).
  - bm2 dropped (softmax shift-invariant).
"""
from contextlib import ExitStack

import ml_dtypes  # noqa: F401
import numpy as np

# ---------------------------------------------------------------------------
# Workaround: this walrus build only accepts one sync-wait per TPB_CTRL
# instruction; split the TileContext exit-drain waits across SP nops.
import concourse.tile as tile
from concourse import mybir
from concourse.vector_clock import ScopedClock


def _patched_drain_and_barrier(self, tick_clock, wait_clock):
    nop0 = self.nc.sync.nop(nofuse=True)
    wait_clock.add_sem_waits(nop0.ins, ScopedClock({None: tick_clock.global_clock}))
    si = nop0.ins.sync_info
    if si is not None and len(si.on_wait) > 1:
        waits = list(si.on_wait)
        nop0.ins.sync_info = mybir.SyncInfo(on_wait=waits[:1], on_update=list(si.on_update))
        for i in range(1, len(waits)):
            nop = self.nc.sync.nop(nofuse=True)
            nop.ins.sync_info = mybir.SyncInfo(on_wait=waits[i:i + 1], on_update=[])
    self.nc.sync.drain()
    self.nc.all_engine_barrier()
    assert self.sems is not None
    popped = self.nc._tile_sem_poison_stack.pop()
    assert popped is self._sem_poison
    self.nc.clear_and_free_semaphores(list(self.sems.allocated().values()))
    self.nc.all_engine_barrier()


tile.TileContext._drain_and_barrier = _patched_drain_and_barrier
# ---------------------------------------------------------------------------

import concourse.bacc as bacc

F32 = mybir.dt.float32
FP16 = mybir.dt.float16
F32R = mybir.dt.float32r

B = 8            # graph pairs (one per core)
N = 512          # nodes per graph
E = 8192         # edges per graph
DIN = 128
DH = 64
R = 32
NB = 4
STEPS = 2
N_CORES = 8

# pblob32 column layout
PB_W2R = 0
PB_W2N = 32
PB_B2 = 64
PB_WM1 = 65
PB_WM1N4 = 97
PB_BM1 = 225
PB_RST0 = 226
PB_RST1 = 738
PB32_COLS = 1250

# pblob128 column layout
PB_W1N = 0
PB_W1R = 64
PB_B1 = 128
PB_RSR = 129
PB128_COLS = 129 + STEPS * NB * R

AX = mybir.AxisListType.X
MAX = mybir.AluOpType.max
ADD = mybir.AluOpType.add
MULT = mybir.AluOpType.mult
RELU = mybir.ActivationFunctionType.Relu
EXP = mybir.ActivationFunctionType.Exp
IDENT = mybir.ActivationFunctionType.Identity
COPY = mybir.ActivationFunctionType.Copy


def build_kernel(repeats=1):
    nc = bacc.Bacc("TRN2", dynamic_dma_scratch_size=32768)

    xsT = nc.declare_dram_parameter("xsT", [DIN, N], F32, isOutput=False)
    xtT = nc.declare_dram_parameter("xtT", [DIN, N], F32, isOutput=False)
    A_in = nc.declare_dram_parameter("A_rows", [2, 128, NB * N], F32, isOutput=False)
    PB128 = nc.declare_dram_parameter("pblob128", [128, PB128_COLS], F32, isOutput=False)
    PB32 = nc.declare_dram_parameter("pblob32", [R, PB32_COLS], F32R, isOutput=False)
    W2p = nc.declare_dram_parameter("W2p", [128, 8, 128], FP16, isOutput=False)
    S0 = nc.declare_dram_parameter("S0", [N, N], F32, isOutput=True)
    SL = nc.declare_dram_parameter("SL", [N, N], F32, isOutput=True)

    with tile.TileContext(nc) as tc, ExitStack() as ctx:
        const = ctx.enter_context(tc.tile_pool(name="const", bufs=1))
        work = ctx.enter_context(tc.tile_pool(name="work", bufs=2))
        souts = ctx.enter_context(tc.tile_pool(name="souts", bufs=4))
        r4p = ctx.enter_context(tc.tile_pool(name="r4p", bufs=14))
        pbig = ctx.enter_context(tc.tile_pool(name="pbig", bufs=4, space="PSUM"))
        psmall = ctx.enter_context(tc.tile_pool(name="psmall", bufs=2, space="PSUM"))
        prtT = ctx.enter_context(tc.tile_pool(name="prtT", bufs=2, space="PSUM"))

        # ---- input DMAs: A_s on the ACT queue, the rest on SP in arrival-
        # priority order (A_t after xt, param blobs around them) ----
        A_s = const.tile([128, NB, N], F32, name="A_s")
        nc.scalar.dma_start(A_s[:].rearrange("p b n -> p (b n)"), A_in[0])
        pb128 = const.tile([128, PB128_COLS], F32)
        nc.sync.dma_start(pb128[:], PB128[:])
        xs = const.tile([DIN, N], F32); nc.sync.dma_start(xs[:], xsT[:])
        xt = const.tile([DIN, N], F32); nc.sync.dma_start(xt[:], xtT[:])
        A_t = const.tile([128, NB, N], F32, name="A_t")
        nc.sync.dma_start(A_t[:].rearrange("p b n -> p (b n)"), A_in[1])
        pb32 = const.tile([R, PB32_COLS], F32R)
        nc.sync.dma_start(pb32[:], PB32[:])
        w2p = const.tile([128, 8, 128], FP16)
        nc.sync.dma_start(w2p[:], W2p[:])
        # f32r-rounded copies of A for the precision-tolerant psi_2 aggs
        # (one-time, outside the repeat body; walrus requires a rounding op)
        A_sr_t = const.tile([128, NB, N], F32R, name="A_sr")
        nc.scalar.copy(A_sr_t[:], A_s[:])
        A_tr_t = const.tile([128, NB, N], F32R, name="A_tr")
        nc.vector.tensor_copy(A_tr_t[:], A_t[:])
        A_sr = A_sr_t[:]
        A_tr = A_tr_t[:]

        # named parameter views
        w1n = pb128[:, PB_W1N:PB_W1N + DH]
        w1r = pb128[:, PB_W1R:PB_W1R + DH]
        b1c = pb128[0:DH, PB_B1:PB_B1 + 1]
        rsr = pb128[:, PB_RSR:PB_RSR + STEPS * NB * R].rearrange(
            "p (k b r) -> p k b r", k=STEPS, b=NB)
        w2r = pb32[:, PB_W2R:PB_W2R + R]
        w2n = pb32[:, PB_W2N:PB_W2N + R]
        b2c = pb32[:, PB_B2:PB_B2 + 1]
        wm1 = pb32[:, PB_WM1:PB_WM1 + R]
        wm1n4 = pb32[:, PB_WM1N4:PB_WM1N4 + 128]
        bm1c = pb32[:, PB_BM1:PB_BM1 + 1]
        rst = [pb32[:, PB_RST0:PB_RST0 + N], pb32[:, PB_RST1:PB_RST1 + N]]

        # warm the ACT function table early (costs ~1.3us once)
        actwarm = const.tile([DH, 1], F32)
        nc.scalar.activation(actwarm[:], b1c, RELU)

        # persistent state tiles
        S_hat = const.tile([128, NB, N], F32, name="S_hat")
        mx0 = const.tile([128, NB], F32, name="mx0")     # -(max of S_hat0)
        sh1 = const.tile([128, NB], F32, name="sh1")     # -(shift for step-1 softmax)
        shF = const.tile([128, NB], F32, name="shF")     # -(shift for final softmax)
        Z0 = const.tile([128, NB], F32, name="Z0")
        Z1 = const.tile([128, NB], F32, name="Z1")
        ZF = const.tile([128, NB], F32, name="ZF")
        rz0 = const.tile([128, NB], F32, name="rz0")
        rz1 = const.tile([128, NB], F32, name="rz1")
        rzF = const.tile([128, NB], F32, name="rzF")
        rsp0 = const.tile([128, NB, R], F32R, name="rsp0")
        rsp1 = const.tile([128, NB, R], F32R, name="rsp1")
        A_packed = [const.tile([128, 128], F32, name=f"A_packed{k}") for k in range(STEPS)]
        cT4neg = [const.tile([128, N], FP16, name=f"cT4neg{k}") for k in range(STEPS)]

        S0v = S0.rearrange("(a b) t -> a b t", b=128)
        SLv = SL.rearrange("(a b) t -> a b t", b=128)

        def add_aggT(ps, A, y_rows, cols, stop=True):
            """ps[f, d] += sum_s y[s, f] A^T[s, d] (agg arrives transposed)."""
            for sc in range(NB):
                nc.tensor.matmul(
                    ps[:], lhsT=y_rows[:, sc, cols], rhs=A[:, sc, :],
                    start=False, stop=(stop and sc == NB - 1),
                    skip_group_check=True)
            return ps

        def psi2s_stages(k):
            """o_s -> a -> A_packed for step k (the s-graph side of psi_2).

            Returns a list of stage closures so the caller can spread the
            issue points (per-engine streams dispatch in order; a stage
            issued too early head-of-line-blocks that engine)."""
            box = {}

            def s1():  # PE: y0 = rs^T W2n
                box["ypk"] = psmall.tile([128, NB, R], F32, tag="small",
                                         name=f"y0pk{k}")
                for nb_ in range(NB):
                    nc.tensor.matmul(box["ypk"][:, nb_, :],
                                     lhsT=rst[k][:, nb_ * 128:(nb_ + 1) * 128],
                                     rhs=w2n, start=True, stop=True,
                                     skip_group_check=True)

            def s2():  # DVE: psum -> sbuf
                box["y0rows"] = work.tile([128, NB, R], F32R, tag="y0rows",
                                          name=f"y0rows{k}")
                nc.vector.tensor_copy(box["y0rows"][:], box["ypk"][:])

            def s3():  # PE: o_s psum = W2r^T rs + A_s-agg
                box["osps"] = psmall.tile([R, N], F32, tag="small", name=f"osps{k}")
                nc.tensor.matmul(box["osps"][:], lhsT=w2r, rhs=rst[k], start=True,
                                 stop=False, skip_group_check=True)
                add_aggT(box["osps"], A_sr, box["y0rows"], slice(0, R))

            def s4():  # ACT: relu -> o_sT
                box["o_sT"] = work.tile([R, N], F32R, tag="o_sT", name=f"o_sT{k}")
                nc.scalar.activation(box["o_sT"][:], box["osps"][:], RELU, bias=b2c)

            def s5():  # PE: a psum = Wm1^T o_s
                box["aps"] = psmall.tile([R, N], F32, tag="small", name=f"aps{k}")
                nc.tensor.matmul(box["aps"][:], lhsT=wm1, rhs=box["o_sT"][:],
                                 start=True, stop=True, skip_group_check=True)

            def s6():  # ACT: + bm1 -> aT
                box["aT"] = work.tile([R, N], F32, tag="aT", name=f"aT{k}")
                nc.scalar.activation(box["aT"][:], box["aps"][:], IDENT, bias=bm1c)

            def s7():  # DVE: rearrange into A_packed
                av = box["aT"][:].rearrange("r (m j) -> r j m", j=4)
                for j in range(4):
                    nc.vector.tensor_copy(A_packed[k][R * j:R * (j + 1), :],
                                          av[:, j, :])

            return [s1, s2, s3, s4, s5, s6, s7]

        def psi2t_tail(k, rtT_ps):
            """rt -> o_t -> -c^T(4x) for step k, after rtT accumulation."""
            rtT = work.tile([R, N], F32R, tag="rtT", name=f"rtT{k}")
            nc.scalar.copy(rtT[:], rtT_ps[:])
            y1pk = psmall.tile([128, NB, R], F32, tag="small", name=f"y1pk{k}")
            for nb_ in range(NB):
                nc.tensor.matmul(y1pk[:, nb_, :], lhsT=rtT[:, nb_ * 128:(nb_ + 1) * 128],
                                 rhs=w2n, start=True, stop=True, skip_group_check=True)
            y1rows = work.tile([128, NB, R], F32R, tag="y0rows", name=f"y1rows{k}")
            nc.vector.tensor_copy(y1rows[:], y1pk[:])
            otps = prtT.tile([R, N], F32, tag="rtT", name=f"otps{k}")
            nc.tensor.matmul(otps[:], lhsT=w2r, rhs=rtT[:], start=True, stop=False,
                             skip_group_check=True)
            add_aggT(otps, A_tr, y1rows, slice(0, R))
            o_tT = work.tile([R, N], F32R, tag="o_sT", name=f"o_tT{k}")
            nc.scalar.activation(o_tT[:], otps[:], RELU, bias=b2c)
            cps4 = pbig.tile([128, N], F32, tag="big", name=f"cps4_{k}")
            nc.tensor.matmul(cps4[:], lhsT=wm1n4, rhs=o_tT[:], start=True, stop=True,
                             skip_group_check=True)
            nc.scalar.copy(cT4neg[k][:], cps4[:])

        def body():
            # -------- psi_1 --------
            def psi1(xT, A, name):
                ypk = psmall.tile([128, NB, DH], F32, tag="small", name=f"y{name}pk")
                for nb_ in range(NB):
                    nc.tensor.matmul(ypk[:, nb_, :], lhsT=xT[:, nb_ * 128:(nb_ + 1) * 128],
                                     rhs=w1n, start=True, stop=True, skip_group_check=True)
                y_rows = work.tile([128, NB, DH], F32, tag=f"y{name}_rows",
                                   name=f"y{name}_rows")
                nc.vector.tensor_copy(y_rows[:], ypk[:])
                hps = pbig.tile([DH, N], F32, tag="big", name=f"h{name}ps")
                nc.tensor.matmul(hps[:], lhsT=w1r, rhs=xT[:], start=True, stop=False,
                                 skip_group_check=True)
                add_aggT(hps, A, y_rows, slice(0, DH))
                # h kept fp32: the S_hat0 contraction amplifies h/matmul error
                # by ~|S_hat| (exp sensitivity), f32r there costs ~1.6e-2 rel
                h = work.tile([DH, N], F32, tag=f"h{name}", name=f"h{name}")
                nc.scalar.activation(h[:], hps[:], RELU, bias=b1c)
                return h

            h_s = psi1(xs, A_s, "s")
            h_t = psi1(xt, A_t, "t")

            # -------- S_hat0 + softmax0 + rt0, per block --------
            S_exp0 = work.tile([128, NB, N], F32R, tag="S_exp", name="S_exp0")
            rtT0_ps = prtT.tile([R, N], F32, tag="rtT", name="rtT0ps")
            for sb in range(NB):
                ps = pbig.tile([128, N], F32, tag="big", name=f"sh0{sb}")
                nc.tensor.matmul(ps[:], lhsT=h_s[:, sb * 128:(sb + 1) * 128],
                                 rhs=h_t[:], start=True, stop=True, skip_group_check=True)
                # copy S_hat to SBUF (alternate DVE/ACT), then reduce and
                # exp from SBUF (PSUM-sourced ACT/DVE ops are slow on HW)
                if sb % 2 == 0:
                    nc.scalar.copy(S_hat[:, sb, :], ps[:])
                else:
                    nc.vector.tensor_copy(S_hat[:, sb, :], ps[:])
                nc.vector.tensor_reduce(mx0[:, sb:sb + 1], S_hat[:, sb, :], axis=AX,
                                        op=MAX, negate=True)
                nc.scalar.activation(S_exp0[:, sb, :], S_hat[:, sb, :], EXP,
                                     bias=mx0[:, sb:sb + 1],
                                     accum_out=Z0[:, sb:sb + 1])
                nc.vector.reciprocal(rz0[:, sb:sb + 1], Z0[:, sb:sb + 1])
                nc.vector.tensor_scalar(
                    out=rsp0[:, sb, :], in0=rsr[:, 0, sb, :],
                    scalar1=rz0[:, sb:sb + 1], scalar2=None, op0=MULT)

            # psi_2 s-side for step 0 (overlaps softmax0 on PE)
            for stage in psi2s_stages(0):
                stage()

            for sb in range(NB):
                nc.tensor.matmul(rtT0_ps[:], lhsT=rsp0[:, sb, :], rhs=S_exp0[:, sb, :],
                                 start=(sb == 0), stop=(sb == NB - 1),
                                 skip_group_check=True)
                # S0 output: exp * rz (split DVE/ACT), then DMA out
                sb_t = souts.tile([128, N], F32, tag="Sout", name=f"S0b{sb}")
                if sb % 2 == 0:
                    nc.vector.tensor_scalar(out=sb_t[:], in0=S_exp0[:, sb, :],
                                            scalar1=rz0[:, sb:sb + 1], scalar2=None,
                                            op0=MULT)
                else:
                    nc.scalar.activation(sb_t[:], S_exp0[:, sb, :], COPY,
                                         scale=rz0[:, sb:sb + 1])
                nc.sync.dma_start(S0v[sb], sb_t[:])

            psi2t_tail(0, rtT0_ps)

            # -------- consensus steps: mlp loop with interleaved chains --------
            S_exp1 = work.tile([128, NB, N], F32R, tag="S_exp", name="S_exp1")
            acc1_box = []

            def chain_a(k, b):
                """S_hat[b] += mlp_ps[b]; -(true row max) from SBUF (DVE)."""
                sh = sh1 if k == 0 else shF
                nc.vector.tensor_tensor(out=S_hat[:, b, :], in0=S_hat[:, b, :],
                                        in1=mlp_ps[b][:], op=ADD)
                nc.vector.tensor_reduce(sh[:, b:b + 1], S_hat[:, b, :], axis=AX,
                                        op=MAX, negate=True)

            def chain_b(k, b):
                if k == 0:
                    nc.scalar.activation(S_exp1[:, b, :], S_hat[:, b, :], EXP,
                                         bias=sh1[:, b:b + 1],
                                         accum_out=Z1[:, b:b + 1])
                    nc.vector.reciprocal(rz1[:, b:b + 1], Z1[:, b:b + 1])
                    nc.vector.tensor_scalar(
                        out=rsp1[:, b, :], in0=rsr[:, 1, b, :],
                        scalar1=rz1[:, b:b + 1], scalar2=None, op0=MULT)
                else:
                    sle = work.tile([128, N], F32, tag="SLe", name=f"SLe{b}")
                    nc.scalar.activation(sle[:], S_hat[:, b, :], EXP,
                                         bias=shF[:, b:b + 1],
                                         accum_out=ZF[:, b:b + 1])
                    nc.vector.reciprocal(rzF[:, b:b + 1], ZF[:, b:b + 1])
                    slb = souts.tile([128, N], F32, tag="Sout", name=f"SLb{b}")
                    nc.scalar.activation(slb[:], sle[:], COPY,
                                         scale=rzF[:, b:b + 1])
                    nc.sync.dma_start(SLv[b], slb[:])

            def chain_c(k, b):
                if k == 0:
                    nc.tensor.matmul(acc1_box[0][:], lhsT=rsp1[:, b, :],
                                     rhs=S_exp1[:, b, :], start=(b == 0),
                                     stop=(b == NB - 1), skip_group_check=True)

            for k in range(STEPS):
                if k == 0:
                    acc1_box.append(prtT.tile([R, N], F32, tag="rtT",
                                              name="rtT1ps"))
                    # deferred psi_2 s-side for step 1, staged across mlp0
                    stages1 = psi2s_stages(1)
                    # (issue m, stage): PE stages early, consumers offset past
                    # the producing engine's ring lag (~12 m's)
                    stage_at = {16: stages1[0], 34: stages1[1], 36: stages1[2],
                                38: stages1[3], 41: stages1[4], 44: stages1[5],
                                72: stages1[6]}
                else:
                    stage_at = {}
                mlp_ps = [pbig.tile([128, N], F32, tag="big", name=f"mlp{k}{b_}")
                          for b_ in range(NB)]
                for m in range(128):
                    r4 = r4p.tile([128, N], FP16, tag="r4")
                    r8 = m % 8
                    if r8 in (3, 7):
                        nc.scalar.activation(r4[:], cT4neg[k][:], RELU,
                                             bias=A_packed[k][:, m:m + 1])
                    else:
                        nc.vector.tensor_scalar(
                            out=r4[:], in0=cT4neg[k][:],
                            scalar1=A_packed[k][:, m:m + 1], scalar2=0.0,
                            op0=ADD, op1=MAX)
                    blk, grp, v = m // 32, (m // 8) % 4, m % 8
                    if grp < 2:
                        nc.tensor.matmul(mlp_ps[blk][32 * grp:32 * (grp + 1), :],
                                         lhsT=w2p[:, v, 0:32], rhs=r4[:],
                                         start=(v == 0), stop=(v == 7))
                    elif grp == 2:
                        nc.tensor.matmul(mlp_ps[blk][64:128, :],
                                         lhsT=w2p[:, v, 0:64], rhs=r4[:],
                                         start=(v == 0), stop=False)
                    else:
                        nc.tensor.matmul(mlp_ps[blk][64:128, :],
                                         lhsT=w2p[:, v, 64:128], rhs=r4[:],
                                         start=False, stop=(v == 7))
                    if m in stage_at:
                        stage_at[m]()
                    # interleaved per-block chains
                    if m >= 32:
                        mb, off = (m // 32) - 1, m % 32
                        if off == 8:
                            chain_a(k, mb)
                        elif off == 14:
                            chain_b(k, mb)
                        elif off == 18:
                            chain_c(k, mb)
                # tail: last block's chain
                chain_a(k, NB - 1)
                chain_b(k, NB - 1)
                chain_c(k, NB - 1)
                if k == 0:
                    psi2t_tail(1, acc1_box[0])

        if repeats == 1:
            body()
        else:
            with tc.For_i(0, repeats, 1):
                body()

    nc.compile()
    return nc


def prep_core_inputs(g, inp):
    n0 = g * N
    xs = np.asarray(inp["x_s"][n0:n0 + N], np.float32)
    xt = np.asarray(inp["x_t"][n0:n0 + N], np.float32)
    es = np.asarray(inp["edge_index_s"][:, g * E:(g + 1) * E]).astype(np.int64) - n0
    et = np.asarray(inp["edge_index_t"][:, g * E:(g + 1) * E]).astype(np.int64) - n0
    eas = np.asarray(inp["edge_attr_s"][g * E:(g + 1) * E, 0], np.float64)
    eat = np.asarray(inp["edge_attr_t"][g * E:(g + 1) * E, 0], np.float64)
    rs = np.asarray(inp["r_s_all"][:, g], np.float32)

    def a_build(edges, ea):
        at = np.bincount(edges[0] * N + edges[1], weights=ea, minlength=N * N)
        at = at.astype(np.float32).reshape(NB, 128, N).transpose(1, 0, 2)
        return np.ascontiguousarray(at.reshape(128, NB * N))

    w2 = np.asarray(inp["Wm2"], np.float32)[:, 0]
    w2p = np.zeros((128, 8, 128), np.float32)
    for v in range(8):
        for j in range(4):
            w2p[R * j:R * (j + 1), v, 4 * v + j] = w2
            w2p[R * j:R * (j + 1), v, 96 + 4 * v + j] = w2

    pb128 = np.zeros((128, PB128_COLS), np.float32)
    pb128[:, PB_W1N:PB_W1N + DH] = np.asarray(inp["W1_nbr"], np.float32)
    pb128[:, PB_W1R:PB_W1R + DH] = np.asarray(inp["W1_root"], np.float32)
    pb128[0:DH, PB_B1] = np.asarray(inp["b1"], np.float32)
    rsr = rs.reshape(STEPS, NB, 128, R).transpose(2, 0, 1, 3)  # [128, k, b, R]
    pb128[:, PB_RSR:] = rsr.reshape(128, STEPS * NB * R)

    pb32 = np.zeros((R, PB32_COLS), np.float32)
    pb32[:, PB_W2R:PB_W2R + R] = np.asarray(inp["W2_root"], np.float32)
    pb32[:, PB_W2N:PB_W2N + R] = np.asarray(inp["W2_nbr"], np.float32)
    pb32[:, PB_B2] = np.asarray(inp["b2"], np.float32)
    wm1 = np.asarray(inp["Wm1"], np.float32)
    pb32[:, PB_WM1:PB_WM1 + R] = wm1
    pb32[:, PB_WM1N4:PB_WM1N4 + 128] = np.tile(-wm1, (1, 4))
    pb32[:, PB_BM1] = np.asarray(inp["bm1"], np.float32)
    rstv = rs.transpose(0, 2, 1)  # [STEPS, R, N]
    pb32[:, PB_RST0:PB_RST0 + N] = rstv[0]
    pb32[:, PB_RST1:PB_RST1 + N] = rstv[1]

    return {
        "xsT": np.ascontiguousarray(xs.T),
        "xtT": np.ascontiguousarray(xt.T),
        "A_rows": np.stack([a_build(es, eas), a_build(et, eat)]),
        "pblob128": pb128,
        "pblob32": pb32,
        "W2p": w2p.astype(np.float16),
    }


_NC_CACHE = {}


def _get_nc(repeats=1):
    if repeats not in _NC_CACHE:
        _NC_CACHE[repeats] = build_kernel(repeats)
    return _NC_CACHE[repeats]


def kernel(**inputs):
    from concourse.bass_utils import run_bass_kernel_spmd
    nc = _get_nc(1)
    in_maps = [prep_core_inputs(g, inputs) for g in range(B)]
    res = run_bass_kernel_spmd(nc, in_maps, core_ids=list(range(N_CORES)))
    S0 = np.stack([res.results[g]["S0"] for g in range(B)])
    SL = np.stack([res.results[g]["SL"] for g in range(B)])
    return S0, SL
